# revision 12
# baseline (speedup 1.0000x reference)
"""InvBlock kernel for 8x TRN2 NeuronCores — fp8 DoubleRow version.

Math (per reference):
  u = x[:, :h], v = x[:, h:]            (h = 2048, B = 16384)
  v_mid = tanh(u @ W1.T + b1)           [B, 4096]
  v_new = v + 0.1 * (v_mid @ W1)        [B, 2048]
  u_mid = tanh(v_new @ W0.T + b0)       [B, 4096]
  u_new = u - 0.1 * (u_mid @ W0)        [B, 2048]
  out   = concat(u_new, v_new)          [B, 4096]

Strategy: data-parallel over batch (2048 rows/core, 8 cores), weights
replicated.  All four matmuls run in fp8(e4m3) with
perf_mode=DoubleRow: each MATMUL contracts K=256 (two 128-row planes,
2 MACs/cell/cycle) into a [128, 512] f32 PSUM bank, ~1.77x the bf16
MM rate.  Weights are pre-scaled by 16 on the host so their values sit
in e4m3's normal range; the 1/16 descale folds into the tanh input
scale (stages A/C) and the 0.1 residual step constant (stages B/D).

Layouts (per core, feature-major / transposed activations):
  activations: [128 part, 2 planes, 2048 batch] fp8 tiles, one per
    256-feature group; plane j holds features kt*256 + j*128 + p.
  weights: per 128-wide output tile mt: [128 ki, KT, 2 j, 128 mi] fp8
    where element = L[kt*256 + j*128 + ki, mt*128 + mi], L the
    already-transposed lhsT matrix (W1.T / W1 / W0.T / W0, x16).

Single pass over the full 2048 batch columns (4 PSUM chunks of 512):
  A: psum[mt] = sum_kt WA[mt,kt]^DR @ u8[kt]   -> tanh(psum/16+b1) -> vmid fp8
  B: psum[mt] = sum_kt WB[mt,kt]^DR @ vmid[kt] -> vnew = v + 0.1/16*psum
     (f32 out to HBM; fp8 copy kept for C)
  C: psum[mt] = sum_kt WC[mt,kt]^DR @ vnew8[kt]-> tanh(psum/16+b0) -> umid fp8
  D: psum[mt] = sum_kt WD[mt,kt]^DR @ umid[kt] -> unew = u - 0.1/16*psum
"""

import numpy as np
import ml_dtypes

import concourse.bacc as bacc
import concourse.mybir as mybir
import concourse.tile as tile
from concourse.bass_utils import run_bass_kernel_spmd
from concourse import bass

FP8 = ml_dtypes.float8_e4m3   # TRN fp8e4: max normal +-240

N_CORES = 8
B = 16384
H = 2048          # h
H2 = 4096         # 2h
BLOC = B // N_CORES   # 2048 batch rows per core
P = 128
F = BLOC          # all 2048 batch columns in one pass
CH = 512          # matmul moving free dim per plane (one PSUM bank of f32)
NCH = F // CH     # 4
KT_A = H // 256   # 8   DoubleRow k-tiles, stages A/C (contract over h)
KT_B = H2 // 256  # 16  DoubleRow k-tiles, stages B/D (contract over 2h)
MT_A = H2 // P    # 32  output tiles, stages A/C
MT_B = H // P     # 16  output tiles, stages B/D
STEP = 0.1
WSCALE = 16.0     # weights pre-scaled by this; descale folded into consts

_CACHE = {}


def _build():
    nc = bacc.Bacc("TRN2", target_bir_lowering=False, debug=False,
                   num_devices=N_CORES)
    dt = mybir.dt
    DR = mybir.MatmulPerfMode.DoubleRow

    u8_d = nc.dram_tensor("u8", [KT_A, P, 2, F], dt.float8e4, kind="ExternalInput")
    uT32_d = nc.dram_tensor("uT32", [H, BLOC], dt.float32, kind="ExternalInput")
    vT32_d = nc.dram_tensor("vT32", [H, BLOC], dt.float32, kind="ExternalInput")
    WA_d = nc.dram_tensor("WA", [MT_A, P, KT_A, 2, P], dt.float8e4, kind="ExternalInput")
    WB_d = nc.dram_tensor("WB", [MT_B, P, KT_B, 2, P], dt.float8e4, kind="ExternalInput")
    WC_d = nc.dram_tensor("WC", [MT_A, P, KT_A, 2, P], dt.float8e4, kind="ExternalInput")
    WD_d = nc.dram_tensor("WD", [MT_B, P, KT_B, 2, P], dt.float8e4, kind="ExternalInput")
    b0_d = nc.dram_tensor("b0t", [P, MT_A], dt.float32, kind="ExternalInput")
    b1_d = nc.dram_tensor("b1t", [P, MT_A], dt.float32, kind="ExternalInput")
    unewT_d = nc.dram_tensor("unewT", [H, BLOC], dt.float32, kind="ExternalOutput")
    vnewT_d = nc.dram_tensor("vnewT", [H, BLOC], dt.float32, kind="ExternalOutput")

    Tanh = mybir.ActivationFunctionType.Tanh
    mult = mybir.AluOpType.mult
    add = mybir.AluOpType.add

    # round-robin DMA issue across two queues to halve issue serialization
    _dma_rr = [0]

    def dma(out, in_):
        eng = nc.sync if _dma_rr[0] % 2 == 0 else nc.gpsimd
        _dma_rr[0] += 1
        eng.dma_start(out=out, in_=in_)

    with tile.TileContext(nc) as tc:
        with (
            # 24 fp8 activation slots [128, 2, 2048]: 8 u8 + 16 vmid, then
            # recycled as 8 vnew8 + 16 umid (96 KiB/partition)
            tc.tile_pool(name="acts", bufs=24) as p_acts,
            tc.tile_pool(name="wt", bufs=3) as p_wt,
            tc.tile_pool(name="res", bufs=3) as p_res,
            tc.tile_pool(name="outp", bufs=3) as p_out,
            tc.tile_pool(name="bias", bufs=1) as p_bias,
            tc.tile_pool(name="ps", bufs=8, space=bass.MemorySpace.PSUM) as p_ps,
        ):
            chunk = [bass.ds(c * CH, CH) for c in range(NCH)]

            # first weight tile before anything else so PE can start ASAP
            wt0 = p_wt.tile([P, KT_B, 2, P], dt.float8e4, tag="wt")
            nc.sync.dma_start(out=wt0[:, :KT_A], in_=WA_d[0])

            b0_sb = p_bias.tile([P, MT_A], dt.float32, tag="b0")
            b1_sb = p_bias.tile([P, MT_A], dt.float32, tag="b1")
            nc.gpsimd.dma_start(out=b0_sb[:], in_=b0_d[:])
            nc.gpsimd.dma_start(out=b1_sb[:], in_=b1_d[:])

            def mm_group(wt, kt_n, rhs_tiles, pss):
                """kt-outer / chunk-inner: one DoubleRow weight load serves
                four 512-wide moving passes (LDWEIGHTS fully hidden)."""
                for kt in range(kt_n):
                    for ch in range(NCH):
                        nc.tensor.matmul(pss[ch][:], wt[:, kt],
                                         rhs_tiles[kt][:, :, chunk[ch]],
                                         start=(kt == 0), stop=(kt == kt_n - 1),
                                         perf_mode=DR, skip_group_check=True)

            # ---- stage A: vmid = tanh((WA^DR @ u8)/16 + b1) ----
            # u8 is the PE-ramp critical path: keep it off the slow software
            # DGE (gpsimd) — alternate the two hardware DGE queues, and split
            # each tile into column halves so the first matmuls start sooner
            HF = F // 2
            u8t = []
            for kt in range(KT_A):
                t = p_acts.tile([P, 2, F], dt.float8e4, tag="acts", name="u8t")
                eng = nc.scalar if kt % 2 == 0 else nc.sync
                eng.dma_start(out=t[:, :, 0:HF], in_=u8_d[kt][:, :, 0:HF])
                u8t.append(t)
            for kt in range(KT_A):
                eng = nc.scalar if kt % 2 == 0 else nc.sync
                eng.dma_start(out=u8t[kt][:, :, HF:F], in_=u8_d[kt][:, :, HF:F])

            # two half-width passes over the batch: the first pass only needs
            # the first halves of u8, halving the bytes the PE ramp waits on
            # (WA streams twice — DMA has ~3x headroom)
            vmid = []
            for half in range(2):
                chs = (0, 1) if half == 0 else (2, 3)
                for mt in range(MT_A):
                    if half == 0 and mt == 0:
                        wt = wt0[:, :KT_A]
                    else:
                        wtf = p_wt.tile([P, KT_B, 2, P], dt.float8e4, tag="wt",
                                        name="wt")
                        dma(wtf[:, :KT_A], WA_d[mt])
                        wt = wtf[:, :KT_A]
                    if half == 0 and mt % 2 == 0:
                        vm = p_acts.tile([P, 2, F], dt.float8e4, tag="acts",
                                         name="vmid")
                        vmid.append(vm)
                    pss = [p_ps.tile([P, CH], dt.float32, tag="ps", name="ps")
                           for _ in chs]
                    for kt in range(KT_A):
                        for i, ch in enumerate(chs):
                            nc.tensor.matmul(pss[i][:], wt[:, kt],
                                             u8t[kt][:, :, chunk[ch]],
                                             start=(kt == 0),
                                             stop=(kt == KT_A - 1),
                                             perf_mode=DR,
                                             skip_group_check=True)
                    for i, ch in enumerate(chs):
                        nc.scalar.activation(vmid[mt // 2][:, mt % 2, chunk[ch]],
                                             pss[i][:], Tanh,
                                             bias=b1_sb[:, mt:mt + 1],
                                             scale=1.0 / WSCALE)

            # ---- stage B: vnew = v + 0.1/16 * (WB^DR @ vmid) ----
            vnew8 = []
            for mt in range(MT_B):
                wt = p_wt.tile([P, KT_B, 2, P], dt.float8e4, tag="wt", name="wt")
                dma(wt[:], WB_d[mt])
                vt = p_res.tile([P, F], dt.float32, tag="res", name="vt")
                dma(vt[:], vT32_d[mt * P:(mt + 1) * P, :])
                of = p_out.tile([P, F], dt.float32, tag="outp", name="of")
                if mt % 2 == 0:
                    vn = p_acts.tile([P, 2, F], dt.float8e4, tag="acts", name="vnew8")
                    vnew8.append(vn)
                pss = [p_ps.tile([P, CH], dt.float32, tag="ps", name="ps")
                       for _ in range(NCH)]
                mm_group(wt, KT_B, vmid, pss)
                for ch in range(NCH):
                    nc.vector.scalar_tensor_tensor(of[:, chunk[ch]], pss[ch][:],
                                                   STEP / WSCALE, vt[:, chunk[ch]],
                                                   op0=mult, op1=add)
                    nc.vector.tensor_copy(vnew8[mt // 2][:, mt % 2, chunk[ch]],
                                          of[:, chunk[ch]])
                dma(vnewT_d[mt * P:(mt + 1) * P, :], of[:])

            # ---- stage C: umid = tanh((WC^DR @ vnew8)/16 + b0) ----
            umid = []
            for mt in range(MT_A):
                wtf = p_wt.tile([P, KT_B, 2, P], dt.float8e4, tag="wt", name="wt")
                dma(wtf[:, :KT_A], WC_d[mt])
                wt = wtf[:, :KT_A]
                if mt % 2 == 0:
                    um = p_acts.tile([P, 2, F], dt.float8e4, tag="acts", name="umid")
                    umid.append(um)
                pss = [p_ps.tile([P, CH], dt.float32, tag="ps", name="ps")
                       for _ in range(NCH)]
                mm_group(wt, KT_A, vnew8, pss)
                for ch in range(NCH):
                    nc.scalar.activation(umid[mt // 2][:, mt % 2, chunk[ch]],
                                         pss[ch][:], Tanh,
                                         bias=b0_sb[:, mt:mt + 1],
                                         scale=1.0 / WSCALE)

            # ---- stage D: unew = u - 0.1/16 * (WD^DR @ umid) ----
            for mt in range(MT_B):
                wt = p_wt.tile([P, KT_B, 2, P], dt.float8e4, tag="wt", name="wt")
                dma(wt[:], WD_d[mt])
                ut = p_res.tile([P, F], dt.float32, tag="res", name="ut")
                dma(ut[:], uT32_d[mt * P:(mt + 1) * P, :])
                of = p_out.tile([P, F], dt.float32, tag="outp", name="of")
                pss = [p_ps.tile([P, CH], dt.float32, tag="ps", name="ps")
                       for _ in range(NCH)]
                mm_group(wt, KT_B, umid, pss)
                for ch in range(NCH):
                    nc.vector.scalar_tensor_tensor(of[:, chunk[ch]], pss[ch][:],
                                                   -STEP / WSCALE, ut[:, chunk[ch]],
                                                   op0=mult, op1=add)
                    # per-chunk store shortens the post-last-matmul tail;
                    # hardware DGE queues only (scalar is idle in stage D)
                    eng = nc.sync if ch % 2 == 0 else nc.scalar
                    eng.dma_start(out=unewT_d[mt * P:(mt + 1) * P, chunk[ch]],
                                  in_=of[:, chunk[ch]])

    nc.compile()
    return nc


def _get_nc():
    if "nc" not in _CACHE:
        _CACHE["nc"] = _build()
    return _CACHE["nc"]


def _wkey(W0, b0, W1, b1):
    import hashlib
    h = hashlib.sha1()
    for a in (W0[::257, ::63], b0[::97], W1[::257, ::63], b1[::97]):
        h.update(np.ascontiguousarray(a).tobytes())
    return h.hexdigest()


def _dr_tiles(L):
    """DoubleRow lhsT tiles: [mt, ki, kt, j, mi] = L[kt*256 + j*128 + ki,
    mt*128 + mi], L fp8 [K, M]."""
    K, M = L.shape
    return np.ascontiguousarray(
        L.reshape(K // 256, 2, P, M // P, P).transpose(3, 2, 0, 1, 4))


def _prep_weights(W0, b0, W1, b1):
    key = _wkey(W0, b0, W1, b1)
    if _CACHE.get("wkey") != key:
        _CACHE.pop("w", None)
        _CACHE["wkey"] = key
    if "w" not in _CACHE:
        W0s = np.clip(W0 * WSCALE, -240, 240).astype(FP8)
        W1s = np.clip(W1 * WSCALE, -240, 240).astype(FP8)
        _CACHE["w"] = {
            "WA": _dr_tiles(np.ascontiguousarray(W1s.T)),
            "WB": _dr_tiles(W1s),
            "WC": _dr_tiles(np.ascontiguousarray(W0s.T)),
            "WD": _dr_tiles(W0s),
            "b0t": np.ascontiguousarray(b0.reshape(MT_A, P).T).astype(np.float32),
            "b1t": np.ascontiguousarray(b1.reshape(MT_A, P).T).astype(np.float32),
        }
    return _CACHE["w"]


def kernel(x, W0, b0, W1, b1, _want_profile=False, _profile_kwargs=None):
    x = np.asarray(x, dtype=np.float32)
    wts = _prep_weights(np.asarray(W0, np.float32), np.asarray(b0, np.float32),
                        np.asarray(W1, np.float32), np.asarray(b1, np.float32))
    nc = _get_nc()

    in_maps = []
    for i in range(N_CORES):
        s = slice(i * BLOC, (i + 1) * BLOC)
        xTs = np.ascontiguousarray(x[s].T)        # [4096, 2048]
        uT32 = xTs[:H]
        vT32 = xTs[H:]
        u8 = np.ascontiguousarray(
            np.clip(uT32, -240, 240).reshape(KT_A, 2, P, F)
            .transpose(0, 2, 1, 3)).astype(FP8)
        in_maps.append({
            "u8": u8,
            "uT32": uT32,
            "vT32": vT32,
            **wts,
        })

    kwargs = dict(_profile_kwargs or {})
    res = run_bass_kernel_spmd(nc, in_maps, core_ids=list(range(N_CORES)),
                               trace=_want_profile, **kwargs)

    out = np.empty((B, H2), np.float32)
    for i in range(N_CORES):
        s = slice(i * BLOC, (i + 1) * BLOC)
        out[s, :H] = res.results[i]["unewT"].T
        out[s, H:] = res.results[i]["vnewT"].T
    if _want_profile:
        return out, res
    return out


# revision 13
# speedup vs baseline: 1.0049x; 1.0049x over previous
"""InvBlock kernel for 8x TRN2 NeuronCores — fp8 DoubleRow version.

Math (per reference):
  u = x[:, :h], v = x[:, h:]            (h = 2048, B = 16384)
  v_mid = tanh(u @ W1.T + b1)           [B, 4096]
  v_new = v + 0.1 * (v_mid @ W1)        [B, 2048]
  u_mid = tanh(v_new @ W0.T + b0)       [B, 4096]
  u_new = u - 0.1 * (u_mid @ W0)        [B, 2048]
  out   = concat(u_new, v_new)          [B, 4096]

Strategy: data-parallel over batch (2048 rows/core, 8 cores), weights
replicated.  All four matmuls run in fp8(e4m3) with
perf_mode=DoubleRow: each MATMUL contracts K=256 (two 128-row planes,
2 MACs/cell/cycle) into a [128, 512] f32 PSUM bank, ~1.77x the bf16
MM rate.  Weights are pre-scaled by 16 on the host so their values sit
in e4m3's normal range; the 1/16 descale folds into the tanh input
scale (stages A/C) and the 0.1 residual step constant (stages B/D).

Layouts (per core, feature-major / transposed activations):
  activations: [128 part, 2 planes, 2048 batch] fp8 tiles, one per
    256-feature group; plane j holds features kt*256 + j*128 + p.
  weights: per 128-wide output tile mt: [128 ki, KT, 2 j, 128 mi] fp8
    where element = L[kt*256 + j*128 + ki, mt*128 + mi], L the
    already-transposed lhsT matrix (W1.T / W1 / W0.T / W0, x16).

Single pass over the full 2048 batch columns (4 PSUM chunks of 512):
  A: psum[mt] = sum_kt WA[mt,kt]^DR @ u8[kt]   -> tanh(psum/16+b1) -> vmid fp8
  B: psum[mt] = sum_kt WB[mt,kt]^DR @ vmid[kt] -> vnew = v + 0.1/16*psum
     (f32 out to HBM; fp8 copy kept for C)
  C: psum[mt] = sum_kt WC[mt,kt]^DR @ vnew8[kt]-> tanh(psum/16+b0) -> umid fp8
  D: psum[mt] = sum_kt WD[mt,kt]^DR @ umid[kt] -> unew = u - 0.1/16*psum
"""

import numpy as np
import ml_dtypes

import concourse.bacc as bacc
import concourse.mybir as mybir
import concourse.tile as tile
from concourse.bass_utils import run_bass_kernel_spmd
from concourse import bass

FP8 = ml_dtypes.float8_e4m3   # TRN fp8e4: max normal +-240

N_CORES = 8
B = 16384
H = 2048          # h
H2 = 4096         # 2h
BLOC = B // N_CORES   # 2048 batch rows per core
P = 128
F = BLOC          # all 2048 batch columns in one pass
CH = 512          # matmul moving free dim per plane (one PSUM bank of f32)
NCH = F // CH     # 4
KT_A = H // 256   # 8   DoubleRow k-tiles, stages A/C (contract over h)
KT_B = H2 // 256  # 16  DoubleRow k-tiles, stages B/D (contract over 2h)
MT_A = H2 // P    # 32  output tiles, stages A/C
MT_B = H // P     # 16  output tiles, stages B/D
STEP = 0.1
WSCALE = 16.0     # weights pre-scaled by this; descale folded into consts

_CACHE = {}


def _build():
    nc = bacc.Bacc("TRN2", target_bir_lowering=False, debug=False,
                   num_devices=N_CORES)
    dt = mybir.dt
    DR = mybir.MatmulPerfMode.DoubleRow

    u8_d = nc.dram_tensor("u8", [KT_A, P, 2, F], dt.float8e4, kind="ExternalInput")
    uT32_d = nc.dram_tensor("uT32", [H, BLOC], dt.float32, kind="ExternalInput")
    vT32_d = nc.dram_tensor("vT32", [H, BLOC], dt.float32, kind="ExternalInput")
    WA_d = nc.dram_tensor("WA", [MT_A, P, KT_A, 2, P], dt.float8e4, kind="ExternalInput")
    WB_d = nc.dram_tensor("WB", [MT_B, P, KT_B, 2, P], dt.float8e4, kind="ExternalInput")
    WC_d = nc.dram_tensor("WC", [MT_A, P, KT_A, 2, P], dt.float8e4, kind="ExternalInput")
    WD_d = nc.dram_tensor("WD", [MT_B, P, KT_B, 2, P], dt.float8e4, kind="ExternalInput")
    b0_d = nc.dram_tensor("b0t", [P, MT_A], dt.float32, kind="ExternalInput")
    b1_d = nc.dram_tensor("b1t", [P, MT_A], dt.float32, kind="ExternalInput")
    unewT_d = nc.dram_tensor("unewT", [H, BLOC], dt.float32, kind="ExternalOutput")
    vnewT_d = nc.dram_tensor("vnewT", [H, BLOC], dt.float32, kind="ExternalOutput")

    Tanh = mybir.ActivationFunctionType.Tanh
    mult = mybir.AluOpType.mult
    add = mybir.AluOpType.add

    # round-robin DMA issue across two queues to halve issue serialization
    _dma_rr = [0]

    def dma(out, in_):
        eng = nc.sync if _dma_rr[0] % 2 == 0 else nc.gpsimd
        _dma_rr[0] += 1
        eng.dma_start(out=out, in_=in_)

    with tile.TileContext(nc) as tc:
        with (
            # 24 fp8 activation slots [128, 2, 2048]: 8 u8 + 16 vmid, then
            # recycled as 8 vnew8 + 16 umid (96 KiB/partition)
            tc.tile_pool(name="acts", bufs=24) as p_acts,
            tc.tile_pool(name="wt", bufs=3) as p_wt,
            tc.tile_pool(name="res", bufs=3) as p_res,
            tc.tile_pool(name="outp", bufs=3) as p_out,
            tc.tile_pool(name="bias", bufs=1) as p_bias,
            tc.tile_pool(name="ps", bufs=8, space=bass.MemorySpace.PSUM) as p_ps,
        ):
            chunk = [bass.ds(c * CH, CH) for c in range(NCH)]

            # first weight tile before anything else so PE can start ASAP
            wt0 = p_wt.tile([P, KT_B, 2, P], dt.float8e4, tag="wt")
            nc.sync.dma_start(out=wt0[:, :KT_A], in_=WA_d[0])

            b0_sb = p_bias.tile([P, MT_A], dt.float32, tag="b0")
            b1_sb = p_bias.tile([P, MT_A], dt.float32, tag="b1")
            nc.gpsimd.dma_start(out=b0_sb[:], in_=b0_d[:])
            nc.gpsimd.dma_start(out=b1_sb[:], in_=b1_d[:])

            def mm_group(wt, kt_n, rhs_tiles, pss):
                """kt-outer / chunk-inner: one DoubleRow weight load serves
                four 512-wide moving passes (LDWEIGHTS fully hidden)."""
                for kt in range(kt_n):
                    for ch in range(NCH):
                        nc.tensor.matmul(pss[ch][:], wt[:, kt],
                                         rhs_tiles[kt][:, :, chunk[ch]],
                                         start=(kt == 0), stop=(kt == kt_n - 1),
                                         perf_mode=DR, skip_group_check=True)

            # ---- stage A: vmid = tanh((WA^DR @ u8)/16 + b1) ----
            # u8 is the PE-ramp critical path: keep it off the slow software
            # DGE (gpsimd) — alternate the two hardware DGE queues instead
            u8t = []
            for kt in range(KT_A):
                t = p_acts.tile([P, 2, F], dt.float8e4, tag="acts", name="u8t")
                eng = nc.scalar if kt % 2 == 0 else nc.sync
                eng.dma_start(out=t[:], in_=u8_d[kt])
                u8t.append(t)
            vmid = []
            for mt in range(MT_A):
                if mt == 0:
                    wt = wt0[:, :KT_A]
                else:
                    wtf = p_wt.tile([P, KT_B, 2, P], dt.float8e4, tag="wt", name="wt")
                    dma(wtf[:, :KT_A], WA_d[mt])
                    wt = wtf[:, :KT_A]
                if mt % 2 == 0:
                    vm = p_acts.tile([P, 2, F], dt.float8e4, tag="acts", name="vmid")
                    vmid.append(vm)
                pss = [p_ps.tile([P, CH], dt.float32, tag="ps", name="ps")
                       for _ in range(NCH)]
                mm_group(wt, KT_A, u8t, pss)
                for ch in range(NCH):
                    nc.scalar.activation(vmid[mt // 2][:, mt % 2, chunk[ch]],
                                         pss[ch][:], Tanh,
                                         bias=b1_sb[:, mt:mt + 1],
                                         scale=1.0 / WSCALE)

            # ---- stage B: vnew = v + 0.1/16 * (WB^DR @ vmid) ----
            vnew8 = []
            for mt in range(MT_B):
                wt = p_wt.tile([P, KT_B, 2, P], dt.float8e4, tag="wt", name="wt")
                dma(wt[:], WB_d[mt])
                vt = p_res.tile([P, F], dt.float32, tag="res", name="vt")
                dma(vt[:], vT32_d[mt * P:(mt + 1) * P, :])
                of = p_out.tile([P, F], dt.float32, tag="outp", name="of")
                if mt % 2 == 0:
                    vn = p_acts.tile([P, 2, F], dt.float8e4, tag="acts", name="vnew8")
                    vnew8.append(vn)
                pss = [p_ps.tile([P, CH], dt.float32, tag="ps", name="ps")
                       for _ in range(NCH)]
                mm_group(wt, KT_B, vmid, pss)
                for ch in range(NCH):
                    nc.vector.scalar_tensor_tensor(of[:, chunk[ch]], pss[ch][:],
                                                   STEP / WSCALE, vt[:, chunk[ch]],
                                                   op0=mult, op1=add)
                    nc.vector.tensor_copy(vnew8[mt // 2][:, mt % 2, chunk[ch]],
                                          of[:, chunk[ch]])
                dma(vnewT_d[mt * P:(mt + 1) * P, :], of[:])

            # ---- stage C: umid = tanh((WC^DR @ vnew8)/16 + b0) ----
            umid = []
            for mt in range(MT_A):
                wtf = p_wt.tile([P, KT_B, 2, P], dt.float8e4, tag="wt", name="wt")
                dma(wtf[:, :KT_A], WC_d[mt])
                wt = wtf[:, :KT_A]
                if mt % 2 == 0:
                    um = p_acts.tile([P, 2, F], dt.float8e4, tag="acts", name="umid")
                    umid.append(um)
                pss = [p_ps.tile([P, CH], dt.float32, tag="ps", name="ps")
                       for _ in range(NCH)]
                mm_group(wt, KT_A, vnew8, pss)
                for ch in range(NCH):
                    nc.scalar.activation(umid[mt // 2][:, mt % 2, chunk[ch]],
                                         pss[ch][:], Tanh,
                                         bias=b0_sb[:, mt:mt + 1],
                                         scale=1.0 / WSCALE)

            # ---- stage D: unew = u - 0.1/16 * (WD^DR @ umid) ----
            for mt in range(MT_B):
                wt = p_wt.tile([P, KT_B, 2, P], dt.float8e4, tag="wt", name="wt")
                dma(wt[:], WD_d[mt])
                ut = p_res.tile([P, F], dt.float32, tag="res", name="ut")
                dma(ut[:], uT32_d[mt * P:(mt + 1) * P, :])
                of = p_out.tile([P, F], dt.float32, tag="outp", name="of")
                pss = [p_ps.tile([P, CH], dt.float32, tag="ps", name="ps")
                       for _ in range(NCH)]
                mm_group(wt, KT_B, umid, pss)
                for ch in range(NCH):
                    nc.vector.scalar_tensor_tensor(of[:, chunk[ch]], pss[ch][:],
                                                   -STEP / WSCALE, ut[:, chunk[ch]],
                                                   op0=mult, op1=add)
                    # per-chunk store shortens the post-last-matmul tail;
                    # hardware DGE queues only (scalar is idle in stage D)
                    eng = nc.sync if ch % 2 == 0 else nc.scalar
                    eng.dma_start(out=unewT_d[mt * P:(mt + 1) * P, chunk[ch]],
                                  in_=of[:, chunk[ch]])

    nc.compile()
    return nc


def _get_nc():
    if "nc" not in _CACHE:
        _CACHE["nc"] = _build()
    return _CACHE["nc"]


def _wkey(W0, b0, W1, b1):
    import hashlib
    h = hashlib.sha1()
    for a in (W0[::257, ::63], b0[::97], W1[::257, ::63], b1[::97]):
        h.update(np.ascontiguousarray(a).tobytes())
    return h.hexdigest()


def _dr_tiles(L):
    """DoubleRow lhsT tiles: [mt, ki, kt, j, mi] = L[kt*256 + j*128 + ki,
    mt*128 + mi], L fp8 [K, M]."""
    K, M = L.shape
    return np.ascontiguousarray(
        L.reshape(K // 256, 2, P, M // P, P).transpose(3, 2, 0, 1, 4))


def _prep_weights(W0, b0, W1, b1):
    key = _wkey(W0, b0, W1, b1)
    if _CACHE.get("wkey") != key:
        _CACHE.pop("w", None)
        _CACHE["wkey"] = key
    if "w" not in _CACHE:
        W0s = np.clip(W0 * WSCALE, -240, 240).astype(FP8)
        W1s = np.clip(W1 * WSCALE, -240, 240).astype(FP8)
        _CACHE["w"] = {
            "WA": _dr_tiles(np.ascontiguousarray(W1s.T)),
            "WB": _dr_tiles(W1s),
            "WC": _dr_tiles(np.ascontiguousarray(W0s.T)),
            "WD": _dr_tiles(W0s),
            "b0t": np.ascontiguousarray(b0.reshape(MT_A, P).T).astype(np.float32),
            "b1t": np.ascontiguousarray(b1.reshape(MT_A, P).T).astype(np.float32),
        }
    return _CACHE["w"]


def kernel(x, W0, b0, W1, b1, _want_profile=False, _profile_kwargs=None):
    x = np.asarray(x, dtype=np.float32)
    wts = _prep_weights(np.asarray(W0, np.float32), np.asarray(b0, np.float32),
                        np.asarray(W1, np.float32), np.asarray(b1, np.float32))
    nc = _get_nc()

    in_maps = []
    for i in range(N_CORES):
        s = slice(i * BLOC, (i + 1) * BLOC)
        xTs = np.ascontiguousarray(x[s].T)        # [4096, 2048]
        uT32 = xTs[:H]
        vT32 = xTs[H:]
        u8 = np.ascontiguousarray(
            np.clip(uT32, -240, 240).reshape(KT_A, 2, P, F)
            .transpose(0, 2, 1, 3)).astype(FP8)
        in_maps.append({
            "u8": u8,
            "uT32": uT32,
            "vT32": vT32,
            **wts,
        })

    kwargs = dict(_profile_kwargs or {})
    res = run_bass_kernel_spmd(nc, in_maps, core_ids=list(range(N_CORES)),
                               trace=_want_profile, **kwargs)

    out = np.empty((B, H2), np.float32)
    for i in range(N_CORES):
        s = slice(i * BLOC, (i + 1) * BLOC)
        out[s, :H] = res.results[i]["unewT"].T
        out[s, H:] = res.results[i]["vnewT"].T
    if _want_profile:
        return out, res
    return out
